# revision 24
# baseline (speedup 1.0000x reference)
"""Trainium2 Bass kernel for nn_Attention_40475771798025.

Full attention layer: QKV projection + RoPE + GQA causal attention + output
projection. B=2, S=2048, D=4096, H=32 q-heads, KV=8 kv-heads, HD=128.

Sharding: head-parallel tensor parallelism across 8 cores. Core g owns kv-head
g (its 4 q-heads, 1 k-head, 1 v-head) for both batches. The output projection
produces per-core partial sums of the full [T, D] output, summed on the host.

All device tensors are fp16 (host-converted): the PE streams fp16 at the same
1 col/cycle as fp32r but gets standalone LDWEIGHTS with FWL that the PE's
reorder window hides under the previous matmul's stream, removing the
~107ns/matmul self-loading weight bubble fp32r pays. fp16's 10 mantissa bits
keep precision near tf32.

Device kernel per core, per batch:
  A: per 512-token tile, per m in (k,q0,v,q1..3): one PSUM bank accumulates
     all 32 D-chunks (no SBUF re-accumulation passes), evacuated fp32->fp16
     into acc; RoPE (rotation-matrix matmul + DVE/GPSIMD combine) runs right
     after each q/k tile evac; V is PE-transposed to [k-token, HD] layout.
     x and wqkv are host-pre-tiled so every DMA line is >=1KB contiguous;
     wqkv/wo/rope tables load once and stay resident across both batches.
  B: per q-head, per 512-wide q-tile: scores^T tiles [128k, 512q] on PE with
     causal sub-columning (diagonal-straddle tiles only compute columns that
     survive the mask; fully-masked columns are never computed, masked via a
     single shared [128,128] triangular bias on the straddle block), exp on
     ACT (paired non-diagonal tiles -> 1024-wide activations to amortize the
     352-cycle instruction overhead), E@V and all-ones denominator matmuls
     accumulate in PSUM a few tiles behind the scores (hides mask+exp
     latency), DVE reciprocal+mul normalizes into the q slot of acc.
  C: partial out^T-free: per 1024 output cols, per 128-token tile: 4-head
     contraction in one PSUM pair, evacuated fp16 (engines round-robin) and
     DMA'd to a [B, 16, 128, D] layout the host untiles.
"""
import sys
sys.path.insert(0, "/opt/trn_rl_repo")
import numpy as np

B, S, D = 2, 2048, 4096
H, KV, HD = 32, 8, 128
REP = H // KV            # 4 q-heads per core
T = B * S                # 4096 flattened tokens
NCORES = 8
P = 128
NC = D // P              # 32 contraction chunks
QTW, KTW = 512, 128      # q-tile width (psum free dim), k-tile width
NT = S // QTW            # 4 q-tiles per batch
NKT = S // KTW           # 16 k-tiles per batch
MQKV = REP + 2           # 6 m slots: q0..q3, k, v
KSLOT, VSLOT = REP, REP + 1
SCALE = 1.0 / float(np.sqrt(HD))
PIPE = 4                 # phase-B EV pipeline depth (pend entries)
NWQ = 4                  # wo column slices
NQ = D // NWQ            # 1024 output cols per slice
M_ORDER = [KSLOT, 0, VSLOT, 1, 2, 3]

_nc = None


def _build_nc(reps=1, phases="ABC"):
    import concourse.bacc as bacc
    import concourse.mybir as mybir
    import concourse.tile as tile
    from contextlib import ExitStack

    F32 = mybir.dt.float32
    F16 = mybir.dt.float16
    BF16 = mybir.dt.bfloat16
    EXP = mybir.ActivationFunctionType.Exp

    nc = bacc.Bacc("TRN2")
    xr_d = nc.dram_tensor("xr", (B, NT, P, NC * QTW), F16, kind="ExternalInput")
    wq_d = nc.dram_tensor("wqr", (MQKV, P, NC * P), F16, kind="ExternalInput")
    wo_d = nc.dram_tensor("wor", (P, REP * D), F16, kind="ExternalInput")
    cs_d = nc.dram_tensor("cdup", (P, S), F16, kind="ExternalInput")
    sn_d = nc.dram_tensor("sdup", (P, S), F16, kind="ExternalInput")
    pt_d = nc.dram_tensor("pt", (P, P), F16, kind="ExternalInput")
    ones_d = nc.dram_tensor("ones", (P, P), BF16, kind="ExternalInput")
    out_d = nc.dram_tensor("out", (B * NWQ * 2 * NT, P, 4 * QTW), F16,
                           kind="ExternalOutput")

    with tile.TileContext(nc) as tc, ExitStack() as top, \
            nc.allow_low_precision(reason="fp16 softmax intermediates"):
        persist = top.enter_context(tc.tile_pool(name="persist", bufs=1))
        accp = top.enter_context(tc.tile_pool(name="acc", bufs=1))
        vnp = top.enter_context(tc.tile_pool(name="vnat", bufs=1))
        xqp = top.enter_context(tc.tile_pool(name="xq", bufs=2))
        tmpp = top.enter_context(tc.tile_pool(name="tmp", bufs=2))
        recp = top.enter_context(tc.tile_pool(name="rec", bufs=2))
        ep = top.enter_context(tc.tile_pool(name="e", bufs=5))
        obp = top.enter_context(tc.tile_pool(name="ob", bufs=2))

        # k slot first on the scalar queue so phase A's first matmuls gate on
        # ~1MB, not the whole 8.4MB weight load; sync queue carries x tiles.
        wq_s = [persist.tile([P, NC, P], F16, tag=f"wq{m}", name=f"wq{m}")
                for m in range(MQKV)]
        for m in M_ORDER:
            nc.scalar.dma_start(
                wq_s[m][:], wq_d[m].rearrange("p (c i) -> p c i", i=P))
        wo_s = persist.tile([P, REP, D], F16)
        nc.scalar.dma_start(wo_s[:], wo_d[:].rearrange("p (h n) -> p h n", n=D))
        cs_s = persist.tile([P, NT, QTW], F16)
        sn_s = persist.tile([P, NT, QTW], F16)
        nc.scalar.dma_start(cs_s[:], cs_d[:].rearrange("p (n q) -> p n q", q=QTW))
        nc.scalar.dma_start(sn_s[:], sn_d[:].rearrange("p (n q) -> p n q", q=QTW))
        pt_s = persist.tile([P, P], F16)
        ones_s = persist.tile([P, P], BF16)
        nc.scalar.dma_start(pt_s[:], pt_d[:])
        nc.scalar.dma_start(ones_s[:], ones_d[:])

        # acc: [128=HD, m, S]; q0..3 roped in A, overwritten with attention
        # output in B (read by C); k roped. v goes to v_raw (bf16) and is
        # DMA-XBAR-transposed into v_nat [k-token, kt, HD].
        acc = accp.tile([P, MQKV - 1, S], F16)
        v_raw = vnp.tile([P, S], BF16)
        v_nat = vnp.tile([P, NKT, HD], BF16)

        evac_engines = [nc.vector, nc.scalar]

        for _rep in range(reps):
          for b in range(B):
            if "A" in phases:
              with ExitStack() as actx:
                psA = actx.enter_context(
                    tc.tile_pool(name="psA", bufs=2, space="PSUM"))
                psRot = actx.enter_context(
                    tc.tile_pool(name="psRot", bufs=2, space="PSUM"))
                nev = 0
                # rope/transpose for m-group i issues after group i+1's
                # matmuls so the PE never waits on group i's PSUM evac
                deferred = []

                def rope_task(m, tt, accsl):
                    def run():
                        rps = psRot.tile([P, QTW], F32, tag="rot", name="rps")
                        nc.tensor.matmul(rps[:], lhsT=pt_s[:], rhs=accsl,
                                         start=True, stop=True)
                        t1 = tmpp.tile([P, QTW], F16, tag="t1", name="t1")
                        t2 = tmpp.tile([P, QTW], F16, tag="t2", name="t2")
                        nc.gpsimd.tensor_mul(t1[:], accsl, cs_s[:, tt, :])
                        nc.vector.tensor_mul(t2[:], rps[:], sn_s[:, tt, :])
                        nc.vector.tensor_add(accsl, t1[:], t2[:])
                    return run

                for tt in range(NT):
                    tsl = slice(tt * QTW, (tt + 1) * QTW)
                    xq = xqp.tile([P, NC, QTW], F16, tag="xq")
                    xsrc = xr_d[b, tt].rearrange("p (c t) -> p c t", t=QTW)
                    for cg in range(4):   # chunked so matmuls start early
                        csl = slice(cg * 8, (cg + 1) * 8)
                        nc.sync.dma_start(xq[:, csl, :], xsrc[:, csl, :])
                    for m in M_ORDER:
                        ps = psA.tile([P, QTW], F32, tag="pa")
                        for c in range(NC):
                            nc.tensor.matmul(
                                ps[:], lhsT=wq_s[m][:, c, :], rhs=xq[:, c, :],
                                start=(c == 0), stop=(c == NC - 1))
                        while len(deferred) > 1:
                            deferred.pop(0)()
                        accsl = (v_raw[:, tsl] if m == VSLOT
                                 else acc[:, m, tsl])
                        ev = evac_engines[nev % 2]
                        nev += 1
                        if ev is nc.scalar:
                            ev.copy(accsl, ps[:])
                        else:
                            ev.tensor_copy(accsl, ps[:])
                        if m == VSLOT:
                            # v -> [k-token, kt, HD] via DMA XBAR transpose
                            nc.scalar.dma_start_transpose(
                                v_nat[:, tt * 4:(tt + 1) * 4, :], accsl)
                        else:
                            deferred.append(rope_task(m, tt, accsl))
                while deferred:
                    deferred.pop(0)()

            # ---- phase B: attention ----
            if "B" in phases:
              with ExitStack() as bctx:
                psS = bctx.enter_context(
                    tc.tile_pool(name="psS", bufs=2, space="PSUM"))
                psO = bctx.enter_context(
                    tc.tile_pool(name="psO", bufs=2, space="PSUM"))
                psD = bctx.enter_context(
                    tc.tile_pool(name="psD", bufs=2, space="PSUM"))
                # pend carries across (h, qt) tiles so one tile's EV
                # drain interleaves with the next tile's score matmuls;
                # entries: (e_ap, kt, col_off, ctx, is_last)
                pend = []

                def finalize(ctx):
                    rec = recp.tile([P, QTW], F32, tag="r", name="rec")
                    nc.vector.reciprocal(rec[:], ctx["ps_d"][:])
                    nc.vector.tensor_mul(acc[:, ctx["h"], ctx["qsl"]],
                                         ctx["ps_o"][:], rec[:])

                def flush(upto):
                    while len(pend) > upto:
                        e_ap, kt, off, ctx, is_last = pend.pop(0)
                        nkt = ctx["nkt"]
                        nc.tensor.matmul(
                            ctx["ps_o"][:, off:], lhsT=v_nat[:, kt, :],
                            rhs=e_ap,
                            start=(kt == 0), stop=(kt == nkt - 1),
                            skip_group_check=True)
                        nc.tensor.matmul(
                            ctx["ps_d"][:, off:], lhsT=ones_s[:], rhs=e_ap,
                            start=(kt == 0), stop=(kt == nkt - 1),
                            skip_group_check=True)
                        if is_last:
                            finalize(ctx)

                for h in range(REP):
                    for qt in range(NT):
                        qsl = slice(qt * QTW, (qt + 1) * QTW)
                        nkt = 4 * (qt + 1)
                        ds = 4 * qt          # first diagonal-straddle k-tile
                        ctx = {
                            "ps_o": psO.tile([P, QTW], F32, tag="o",
                                             name="ps_o"),
                            "ps_d": psD.tile([P, QTW], F32, tag="d",
                                             name="ps_d"),
                            "nkt": nkt, "h": h, "qsl": qsl,
                        }
                        for p2 in range(ds // 2):   # below-diagonal k, paired
                            ps = psS.tile([P, 2 * QTW], F32, tag="s")
                            for half in range(2):
                                kt = 2 * p2 + half
                                nc.tensor.matmul(
                                    ps[:, half * QTW:(half + 1) * QTW],
                                    lhsT=acc[:, KSLOT,
                                             kt * KTW:(kt + 1) * KTW],
                                    rhs=acc[:, h, qsl],
                                    start=True, stop=True)
                            e = ep.tile([P, 2 * QTW], BF16, tag="e")
                            nc.scalar.activation(e[:], ps[:], EXP, scale=SCALE)
                            pend.append((e[:, :QTW], 2 * p2, 0, ctx, False))
                            pend.append((e[:, QTW:], 2 * p2 + 1, 0, ctx,
                                         False))
                            flush(PIPE)
                        for j in range(4):          # diagonal-straddle k
                            kt = ds + j
                            off = j * KTW
                            w = QTW - off
                            ps = psS.tile([P, QTW], F32, tag="s",
                                          padded_shape=[P, 2 * QTW])
                            nc.tensor.matmul(
                                ps[:, off:],
                                lhsT=acc[:, KSLOT, kt * KTW:(kt + 1) * KTW],
                                rhs=acc[:, h, qt * QTW + off:(qt + 1) * QTW],
                                start=True, stop=True)
                            e = ep.tile([P, QTW], BF16, tag="e",
                                        padded_shape=[P, 2 * QTW])
                            nc.scalar.activation(e[:, :w], ps[:, off:], EXP,
                                                 scale=SCALE)
                            # zero anti-causal half of the straddle block:
                            # keep col k >= partition p  (iota = k - p >= 0)
                            nc.gpsimd.affine_select(
                                e[:, :KTW], e[:, :KTW],
                                pattern=[[1, KTW]],
                                compare_op=mybir.AluOpType.is_ge,
                                fill=0.0, base=0, channel_multiplier=-1)
                            pend.append((e[:, :w], kt, off, ctx, j == 3))
                            flush(PIPE)
                flush(0)

            # ---- phase C: output projection (partial) ----
            if "C" in phases:
              with ExitStack() as cctx:
                psC = cctx.enter_context(
                    tc.tile_pool(name="psC", bufs=2, space="PSUM"))
                nev = 0
                for nq in range(NWQ):
                    for half in range(NQ // QTW):
                        col = nq * NQ + half * QTW
                        for qtb in range(NT):
                            # 4 token tiles batched into one staging tile ->
                            # one DMA with 4KB-contiguous per-partition runs
                            obg = obp.tile([P, 4, QTW], F16, tag="ob",
                                           name="obg")
                            for ti in range(4):
                                tt = 4 * qtb + ti
                                ps = psC.tile([P, QTW], F32, tag="pc",
                                              name="ps_c")
                                for h in range(REP):
                                    nc.tensor.matmul(
                                        ps[:],
                                        lhsT=acc[:, h, tt * P:(tt + 1) * P],
                                        rhs=wo_s[:, h, col:col + QTW],
                                        start=(h == 0), stop=(h == REP - 1))
                                ev = evac_engines[nev % 2]
                                nev += 1
                                if ev is nc.scalar:
                                    ev.copy(obg[:, ti, :], ps[:])
                                else:
                                    ev.tensor_copy(obg[:, ti, :], ps[:])
                            idx = ((b * NWQ + nq) * 2 + half) * NT + qtb
                            nc.sync.dma_start(out_d[idx], obg[:])
    nc.compile()
    return nc


def get_nc():
    global _nc
    if _nc is None:
        _nc = _build_nc()
    return _nc


def make_in_maps(x, freqs_cos, freqs_sin, wq, wk, wv, wo):
    """Host-side prep: fp16 convert, tilings, rope tables, per-core shards."""
    x = np.asarray(x, np.float32)
    fc = np.asarray(freqs_cos, np.float32)
    fs = np.asarray(freqs_sin, np.float32)
    wq = np.asarray(wq, np.float32)
    wk = np.asarray(wk, np.float32)
    wv = np.asarray(wv, np.float32)
    wo = np.asarray(wo, np.float32)

    # xr[b, tt, p, c, t] = x[b, tt*512+t, c*128+p]: 32KB-contiguous lines
    xr = np.ascontiguousarray(
        x.reshape(B, NT, QTW, NC, P).transpose(0, 1, 4, 3, 2)
        .reshape(B, NT, P, NC * QTW).astype(np.float16))
    cdup = np.ascontiguousarray(np.repeat(fc.T, 2, axis=0)).astype(np.float16)
    sdup = np.ascontiguousarray(np.repeat(fs.T, 2, axis=0)).astype(np.float16)
    prot = np.zeros((P, P), np.float32)
    for i in range(P // 2):
        prot[2 * i, 2 * i + 1] = -1.0
        prot[2 * i + 1, 2 * i] = 1.0
    pt = np.ascontiguousarray(prot.T).astype(np.float16)
    import ml_dtypes
    ones = np.ones((P, P), ml_dtypes.bfloat16)

    in_maps = []
    for g in range(NCORES):
        wq_g = wq[g * REP * HD:(g + 1) * REP * HD]
        wk_g = wk[g * HD:(g + 1) * HD]
        wv_g = wv[g * HD:(g + 1) * HD]
        slots = [wq_g[m * P:(m + 1) * P] for m in range(REP)] + [wk_g, wv_g]
        # wqr[m, p, c, i] = W_m[i, c*128+p]
        wqr = np.stack([
            np.ascontiguousarray(
                Wm.T.reshape(NC, P, P).transpose(1, 0, 2).reshape(P, NC * P))
            for Wm in slots]).astype(np.float16)
        # wor[p, h, n] = wo[n, g*512 + h*128 + p]
        wog = wo[:, g * REP * HD:(g + 1) * REP * HD]   # [4096, 512]
        wor = np.ascontiguousarray(
            wog.T.reshape(REP, P, D).transpose(1, 0, 2).reshape(P, REP * D)
        ).astype(np.float16)
        in_maps.append({
            "xr": xr, "wqr": wqr, "wor": wor,
            "cdup": cdup, "sdup": sdup, "pt": pt, "ones": ones,
        })
    return in_maps


def untile_out(acc):
    """Device out layout [(b nq half qtb), p, (ti q)] -> [B, S, D]."""
    acc = acc.reshape(B, NWQ, 2, NT, P, 4, QTW)
    return acc.transpose(0, 3, 5, 4, 1, 2, 6).reshape(B, S, D)


def assemble(results):
    """Sum per-core fp16 partials -> full [B, S, D] fp32."""
    out = np.zeros((B * NWQ * 2 * NT, P, 4 * QTW), np.float32)
    for r in results:
        out += r["out"].astype(np.float32)
    return untile_out(out)


def kernel(x, freqs_cos, freqs_sin, wq, wk, wv, wo):
    from concourse.bass_utils import run_bass_kernel_spmd
    nc = get_nc()
    in_maps = make_in_maps(x, freqs_cos, freqs_sin, wq, wk, wv, wo)
    res = run_bass_kernel_spmd(nc, in_maps, core_ids=list(range(NCORES)))
    return assemble(res.results)


# revision 27
# speedup vs baseline: 1.6879x; 1.6879x over previous
"""Trainium2 Bass kernel for nn_Attention_40475771798025.

Full attention layer: QKV projection + RoPE + GQA causal attention + output
projection. B=2, S=2048, D=4096, H=32 q-heads, KV=8 kv-heads, HD=128.

Sharding: head-parallel tensor parallelism across 8 cores. Core g owns kv-head
g (its 4 q-heads, 1 k-head, 1 v-head) for both batches. The output projection
produces per-core partial sums of the full [T, D] output, summed on the host.

Matmul operands are fp16 (host-converted); the exp output, V, and the all-
ones tile are bf16 (exp of the max logit ~e^18 overflows fp16). fp16 streams
at the same 1 col/cycle as fp32r but gets standalone LDWEIGHTS with FWL that
the PE's reorder window hides under the previous matmul's stream, removing
the ~107ns/matmul self-loading weight bubble fp32r pays.

Device kernel per core, per batch:
  A: per 512-token tile, per m in (k,q0,v,q1..3): one PSUM bank accumulates
     all 32 D-chunks (no SBUF re-accumulation passes), evacuated fp32->fp16
     into acc; RoPE (rotation-matrix matmul + DVE/GPSIMD combine) runs right
     after each q/k tile evac; V is PE-transposed to [k-token, HD] layout.
     x and wqkv are host-pre-tiled so every DMA line is >=1KB contiguous;
     wqkv/wo/rope tables load once and stay resident across both batches.
  B: per q-head, per 512-wide q-tile: scores^T tiles [128k, 512q] on PE with
     causal sub-columning (diagonal-straddle tiles only compute columns that
     survive the mask; the anti-causal straddle half is zeroed post-exp by a
     gpsimd affine_select, off the ACT critical path), exp on ACT (paired
     non-diagonal tiles -> 1024-wide activations amortize the 352-cycle
     instruction overhead), E@V and all-ones denominator matmuls accumulate
     in PSUM a few entries behind the scores via a pend list carried across
     (h, qt) tiles -- one tile's EV drain interleaves with the next tile's
     score matmuls -- and DVE reciprocal+mul normalizes into acc's q slot.
     V reaches its [k-token, HD] layout via a DMA XBAR transpose on the
     scalar queue instead of PE transposes.
  C: partial out^T-free: per 1024 output cols, per 128-token tile: 4-head
     contraction in one PSUM pair, evacuated fp16 (engines round-robin) and
     DMA'd to a [B, 16, 128, D] layout the host untiles.
"""
import sys
sys.path.insert(0, "/opt/trn_rl_repo")
import numpy as np

B, S, D = 2, 2048, 4096
H, KV, HD = 32, 8, 128
REP = H // KV            # 4 q-heads per core
T = B * S                # 4096 flattened tokens
NCORES = 8
P = 128
NC = D // P              # 32 contraction chunks
QTW, KTW = 512, 128      # q-tile width (psum free dim), k-tile width
NT = S // QTW            # 4 q-tiles per batch
NKT = S // KTW           # 16 k-tiles per batch
MQKV = REP + 2           # 6 m slots: q0..q3, k, v
KSLOT, VSLOT = REP, REP + 1
SCALE = 1.0 / float(np.sqrt(HD))
PIPE = 4                 # phase-B EV pipeline depth (pend entries)
NWQ = 4                  # wo column slices
NQ = D // NWQ            # 1024 output cols per slice
M_ORDER = [KSLOT, 0, VSLOT, 1, 2, 3]

_nc = None


def _build_nc(reps=1, phases="ABC"):
    import concourse.bacc as bacc
    import concourse.mybir as mybir
    import concourse.tile as tile
    from contextlib import ExitStack

    F32 = mybir.dt.float32
    F16 = mybir.dt.float16
    BF16 = mybir.dt.bfloat16
    EXP = mybir.ActivationFunctionType.Exp

    nc = bacc.Bacc("TRN2")
    xr_d = nc.dram_tensor("xr", (B, NT, P, NC * QTW), F16, kind="ExternalInput")
    wq_d = nc.dram_tensor("wqr", (MQKV, P, NC * P), F16, kind="ExternalInput")
    wo_d = nc.dram_tensor("wor", (P, REP * D), F16, kind="ExternalInput")
    cs_d = nc.dram_tensor("cdup", (P, S), F16, kind="ExternalInput")
    sn_d = nc.dram_tensor("sdup", (P, S), F16, kind="ExternalInput")
    pt_d = nc.dram_tensor("pt", (P, P), F16, kind="ExternalInput")
    ones_d = nc.dram_tensor("ones", (P, P), BF16, kind="ExternalInput")
    out_d = nc.dram_tensor("out", (B, S // P, P, D), F16, kind="ExternalOutput")

    with tile.TileContext(nc) as tc, ExitStack() as top, \
            nc.allow_low_precision(reason="fp16 softmax intermediates"):
        persist = top.enter_context(tc.tile_pool(name="persist", bufs=1))
        accp = top.enter_context(tc.tile_pool(name="acc", bufs=1))
        vnp = top.enter_context(tc.tile_pool(name="vnat", bufs=1))
        xqp = top.enter_context(tc.tile_pool(name="xq", bufs=2))
        tmpp = top.enter_context(tc.tile_pool(name="tmp", bufs=2))
        recp = top.enter_context(tc.tile_pool(name="rec", bufs=2))
        ep = top.enter_context(tc.tile_pool(name="e", bufs=5))
        obp = top.enter_context(tc.tile_pool(name="ob", bufs=3))

        # k slot first on the scalar queue so phase A's first matmuls gate on
        # ~1MB, not the whole 8.4MB weight load; sync queue carries x tiles.
        wq_s = [persist.tile([P, NC, P], F16, tag=f"wq{m}", name=f"wq{m}")
                for m in range(MQKV)]
        for m in M_ORDER:
            nc.scalar.dma_start(
                wq_s[m][:], wq_d[m].rearrange("p (c i) -> p c i", i=P))
        wo_s = persist.tile([P, REP, D], F16)
        nc.scalar.dma_start(wo_s[:], wo_d[:].rearrange("p (h n) -> p h n", n=D))
        cs_s = persist.tile([P, NT, QTW], F16)
        sn_s = persist.tile([P, NT, QTW], F16)
        nc.scalar.dma_start(cs_s[:], cs_d[:].rearrange("p (n q) -> p n q", q=QTW))
        nc.scalar.dma_start(sn_s[:], sn_d[:].rearrange("p (n q) -> p n q", q=QTW))
        pt_s = persist.tile([P, P], F16)
        ones_s = persist.tile([P, P], BF16)
        nc.scalar.dma_start(pt_s[:], pt_d[:])
        nc.scalar.dma_start(ones_s[:], ones_d[:])

        # acc: [128=HD, m, S]; q0..3 roped in A, overwritten with attention
        # output in B (read by C); k roped. v goes to v_raw (bf16) and is
        # DMA-XBAR-transposed into v_nat [k-token, kt, HD].
        acc = accp.tile([P, MQKV - 1, S], F16)
        v_raw = vnp.tile([P, S], BF16)
        v_nat = vnp.tile([P, NKT, HD], BF16)

        evac_engines = [nc.vector, nc.scalar]

        for _rep in range(reps):
          for b in range(B):
            if "A" in phases:
              with ExitStack() as actx:
                psA = actx.enter_context(
                    tc.tile_pool(name="psA", bufs=2, space="PSUM"))
                psRot = actx.enter_context(
                    tc.tile_pool(name="psRot", bufs=2, space="PSUM"))
                nev = 0
                # rope/transpose for m-group i issues after group i+1's
                # matmuls so the PE never waits on group i's PSUM evac
                deferred = []

                def rope_task(m, tt, accsl):
                    def run():
                        rps = psRot.tile([P, QTW], F32, tag="rot", name="rps")
                        nc.tensor.matmul(rps[:], lhsT=pt_s[:], rhs=accsl,
                                         start=True, stop=True)
                        t1 = tmpp.tile([P, QTW], F16, tag="t1", name="t1")
                        t2 = tmpp.tile([P, QTW], F16, tag="t2", name="t2")
                        nc.gpsimd.tensor_mul(t1[:], accsl, cs_s[:, tt, :])
                        nc.vector.tensor_mul(t2[:], rps[:], sn_s[:, tt, :])
                        nc.vector.tensor_add(accsl, t1[:], t2[:])
                    return run

                for tt in range(NT):
                    tsl = slice(tt * QTW, (tt + 1) * QTW)
                    xq = xqp.tile([P, NC, QTW], F16, tag="xq")
                    xsrc = xr_d[b, tt].rearrange("p (c t) -> p c t", t=QTW)
                    for cg in range(4):   # chunked so matmuls start early
                        csl = slice(cg * 8, (cg + 1) * 8)
                        nc.sync.dma_start(xq[:, csl, :], xsrc[:, csl, :])
                    for m in M_ORDER:
                        ps = psA.tile([P, QTW], F32, tag="pa")
                        for c in range(NC):
                            nc.tensor.matmul(
                                ps[:], lhsT=wq_s[m][:, c, :], rhs=xq[:, c, :],
                                start=(c == 0), stop=(c == NC - 1))
                        while len(deferred) > 1:
                            deferred.pop(0)()
                        accsl = (v_raw[:, tsl] if m == VSLOT
                                 else acc[:, m, tsl])
                        ev = evac_engines[nev % 2]
                        nev += 1
                        if ev is nc.scalar:
                            ev.copy(accsl, ps[:])
                        else:
                            ev.tensor_copy(accsl, ps[:])
                        if m == VSLOT:
                            # v -> [k-token, kt, HD] via DMA XBAR transpose
                            nc.scalar.dma_start_transpose(
                                v_nat[:, tt * 4:(tt + 1) * 4, :], accsl)
                        else:
                            deferred.append(rope_task(m, tt, accsl))
                while deferred:
                    deferred.pop(0)()

            # ---- phase B: attention ----
            if "B" in phases:
              with ExitStack() as bctx:
                psS = bctx.enter_context(
                    tc.tile_pool(name="psS", bufs=2, space="PSUM"))
                psO = bctx.enter_context(
                    tc.tile_pool(name="psO", bufs=2, space="PSUM"))
                psD = bctx.enter_context(
                    tc.tile_pool(name="psD", bufs=2, space="PSUM"))
                # pend carries across (h, qt) tiles so one tile's EV
                # drain interleaves with the next tile's score matmuls;
                # entries: (e_ap, kt, col_off, ctx, is_last)
                pend = []

                def finalize(ctx):
                    rec = recp.tile([P, QTW], F32, tag="r", name="rec")
                    nc.vector.reciprocal(rec[:], ctx["ps_d"][:])
                    nc.vector.tensor_mul(acc[:, ctx["h"], ctx["qsl"]],
                                         ctx["ps_o"][:], rec[:])

                def flush(upto):
                    while len(pend) > upto:
                        e_ap, kt, off, ctx, is_last = pend.pop(0)
                        nkt = ctx["nkt"]
                        nc.tensor.matmul(
                            ctx["ps_o"][:, off:], lhsT=v_nat[:, kt, :],
                            rhs=e_ap,
                            start=(kt == 0), stop=(kt == nkt - 1),
                            skip_group_check=True)
                        nc.tensor.matmul(
                            ctx["ps_d"][:, off:], lhsT=ones_s[:], rhs=e_ap,
                            start=(kt == 0), stop=(kt == nkt - 1),
                            skip_group_check=True)
                        if is_last:
                            finalize(ctx)

                for h in range(REP):
                    for qt in range(NT):
                        qsl = slice(qt * QTW, (qt + 1) * QTW)
                        nkt = 4 * (qt + 1)
                        ds = 4 * qt          # first diagonal-straddle k-tile
                        ctx = {
                            "ps_o": psO.tile([P, QTW], F32, tag="o",
                                             name="ps_o"),
                            "ps_d": psD.tile([P, QTW], F32, tag="d",
                                             name="ps_d"),
                            "nkt": nkt, "h": h, "qsl": qsl,
                        }
                        for p2 in range(ds // 2):   # below-diagonal k, paired
                            ps = psS.tile([P, 2 * QTW], F32, tag="s")
                            for half in range(2):
                                kt = 2 * p2 + half
                                nc.tensor.matmul(
                                    ps[:, half * QTW:(half + 1) * QTW],
                                    lhsT=acc[:, KSLOT,
                                             kt * KTW:(kt + 1) * KTW],
                                    rhs=acc[:, h, qsl],
                                    start=True, stop=True)
                            e = ep.tile([P, 2 * QTW], BF16, tag="e")
                            nc.scalar.activation(e[:], ps[:], EXP, scale=SCALE)
                            pend.append((e[:, :QTW], 2 * p2, 0, ctx, False))
                            pend.append((e[:, QTW:], 2 * p2 + 1, 0, ctx,
                                         False))
                            flush(PIPE)
                        for j in range(4):          # diagonal-straddle k
                            kt = ds + j
                            off = j * KTW
                            w = QTW - off
                            ps = psS.tile([P, QTW], F32, tag="s",
                                          padded_shape=[P, 2 * QTW])
                            nc.tensor.matmul(
                                ps[:, off:],
                                lhsT=acc[:, KSLOT, kt * KTW:(kt + 1) * KTW],
                                rhs=acc[:, h, qt * QTW + off:(qt + 1) * QTW],
                                start=True, stop=True)
                            e = ep.tile([P, QTW], BF16, tag="e",
                                        padded_shape=[P, 2 * QTW])
                            nc.scalar.activation(e[:, :w], ps[:, off:], EXP,
                                                 scale=SCALE)
                            # zero anti-causal half of the straddle block:
                            # keep col k >= partition p  (iota = k - p >= 0)
                            nc.gpsimd.affine_select(
                                e[:, :KTW], e[:, :KTW],
                                pattern=[[1, KTW]],
                                compare_op=mybir.AluOpType.is_ge,
                                fill=0.0, base=0, channel_multiplier=-1)
                            pend.append((e[:, :w], kt, off, ctx, j == 3))
                            flush(PIPE)
                flush(0)

            # ---- phase C: output projection (partial) ----
            if "C" in phases:
              with ExitStack() as cctx:
                psC = cctx.enter_context(
                    tc.tile_pool(name="psC", bufs=2, space="PSUM"))
                nev = 0
                for nq in range(NWQ):
                    nsl = slice(nq * NQ, (nq + 1) * NQ)
                    for tt in range(S // P):
                        ps = psC.tile([P, NQ], F32, tag="pc")
                        for half in range(NQ // QTW):
                            hsl = slice(half * QTW, (half + 1) * QTW)
                            for h in range(REP):
                                nc.tensor.matmul(
                                    ps[:, hsl],
                                    lhsT=acc[:, h, tt * P:(tt + 1) * P],
                                    rhs=wo_s[:, h,
                                             nq * NQ + half * QTW:
                                             nq * NQ + (half + 1) * QTW],
                                    start=(h == 0), stop=(h == REP - 1))
                        ob = obp.tile([P, NQ], F16, tag="ob")
                        ev = evac_engines[nev % 2]
                        nev += 1
                        if ev is nc.scalar:
                            ev.copy(ob[:], ps[:])
                        else:
                            ev.tensor_copy(ob[:], ps[:])
                        nc.sync.dma_start(out_d[b, tt, :, nsl], ob[:])
    nc.compile()
    return nc


def get_nc():
    global _nc
    if _nc is None:
        _nc = _build_nc()
    return _nc


def make_in_maps(x, freqs_cos, freqs_sin, wq, wk, wv, wo):
    """Host-side prep: fp16 convert, tilings, rope tables, per-core shards."""
    x = np.asarray(x, np.float32)
    fc = np.asarray(freqs_cos, np.float32)
    fs = np.asarray(freqs_sin, np.float32)
    wq = np.asarray(wq, np.float32)
    wk = np.asarray(wk, np.float32)
    wv = np.asarray(wv, np.float32)
    wo = np.asarray(wo, np.float32)

    # xr[b, tt, p, c, t] = x[b, tt*512+t, c*128+p]: 32KB-contiguous lines
    xr = np.ascontiguousarray(
        x.reshape(B, NT, QTW, NC, P).transpose(0, 1, 4, 3, 2)
        .reshape(B, NT, P, NC * QTW).astype(np.float16))
    cdup = np.ascontiguousarray(np.repeat(fc.T, 2, axis=0)).astype(np.float16)
    sdup = np.ascontiguousarray(np.repeat(fs.T, 2, axis=0)).astype(np.float16)
    prot = np.zeros((P, P), np.float32)
    for i in range(P // 2):
        prot[2 * i, 2 * i + 1] = -1.0
        prot[2 * i + 1, 2 * i] = 1.0
    pt = np.ascontiguousarray(prot.T).astype(np.float16)
    import ml_dtypes
    ones = np.ones((P, P), ml_dtypes.bfloat16)

    in_maps = []
    for g in range(NCORES):
        wq_g = wq[g * REP * HD:(g + 1) * REP * HD]
        wk_g = wk[g * HD:(g + 1) * HD]
        wv_g = wv[g * HD:(g + 1) * HD]
        slots = [wq_g[m * P:(m + 1) * P] for m in range(REP)] + [wk_g, wv_g]
        # wqr[m, p, c, i] = W_m[i, c*128+p]
        wqr = np.stack([
            np.ascontiguousarray(
                Wm.T.reshape(NC, P, P).transpose(1, 0, 2).reshape(P, NC * P))
            for Wm in slots]).astype(np.float16)
        # wor[p, h, n] = wo[n, g*512 + h*128 + p]
        wog = wo[:, g * REP * HD:(g + 1) * REP * HD]   # [4096, 512]
        wor = np.ascontiguousarray(
            wog.T.reshape(REP, P, D).transpose(1, 0, 2).reshape(P, REP * D)
        ).astype(np.float16)
        in_maps.append({
            "xr": xr, "wqr": wqr, "wor": wor,
            "cdup": cdup, "sdup": sdup, "pt": pt, "ones": ones,
        })
    return in_maps


def untile_out(out):
    """Device out layout [B, S//P, P, D] -> [B, S, D]."""
    return out.reshape(B, S, D)


def assemble(results):
    """Sum per-core fp16 partials -> full [B, S, D] fp32."""
    out = np.zeros((B, S // P, P, D), np.float32)
    for r in results:
        out += r["out"].astype(np.float32)
    return untile_out(out)


def kernel(x, freqs_cos, freqs_sin, wq, wk, wv, wo):
    from concourse.bass_utils import run_bass_kernel_spmd
    nc = get_nc()
    in_maps = make_in_maps(x, freqs_cos, freqs_sin, wq, wk, wv, wo)
    res = run_bass_kernel_spmd(nc, in_maps, core_ids=list(range(NCORES)))
    return assemble(res.results)
